# revision 10
# baseline (speedup 1.0000x reference)
"""Batched CRF Viterbi decode on 8 Trainium2 NeuronCores.

Strategy (data parallel, 32 sentences/core):
  Device forward pass computes the Viterbi DP table fv[t] for every step
  (the O(B*T*L^2) part) and streams it to HBM.  Per step:
    - 4 tiny PE matmuls all-gather fv across the 4 partition j-groups into
      PSUM (broadcast fv[b, :] to every partition that needs it) .
    - one DVE tensor_tensor add: tmp[(g,b),(jl,i)] = fv[b,i] + T[i, g*16+jl]
    - one DVE segmented reduce_max over i -> mx[(g,b), jl]
    - one tiny DVE add of the emission -> fv_t
  Host does the O(B*T*L) backtrace (argmax recompute along the path),
  bit-exact in float32 so paths match the reference argmax tie-breaks.

Layout per core: partition p = g*32 + b  (g = j-group of 16 labels,
b = sentence), free dim inside a step = (jl, i) = (16, 64).
"""

import sys

import numpy as np

for _p in ("/opt/trn_rl_repo",):
    if _p not in sys.path:
        sys.path.insert(0, _p)

B, T, L = 256, 1024, 64
NCORES = 8
BC = B // NCORES  # 32 sentences per core
G, JL = 4, 16  # 4 j-groups x 16 labels
NEG = -10000.0
BOS, EOS = 62, 63
CH = 64  # steps per DMA chunk
NCH = T // CH

_CACHE = {}
LAST_EXEC_NS = None


def _build_bass():
    import concourse.bass as bass
    import concourse.bacc as bacc
    import concourse.mybir as mybir
    from concourse.mybir import AluOpType
    from concourse.tile import TileContext

    f32 = mybir.dt.float32
    nc = bacc.Bacc(None, target_bir_lowering=False)

    xre = nc.dram_tensor("xre", [128, T * JL], f32, kind="ExternalInput")
    # consts concatenated so ONE DMA (one semaphore) loads them all:
    # [texp | lhst | fvinit] = [1024 | 512 | 16]
    CW = JL * L + G * 128 + JL
    consts_d = nc.dram_tensor("consts", [128, CW], f32, kind="ExternalInput")
    fvout_d = nc.dram_tensor("fvout", [128, T * JL], f32, kind="ExternalOutput")

    with TileContext(nc) as tc:
        with tc.tile_pool(name="const", bufs=1) as constp, \
             tc.tile_pool(name="xin", bufs=3) as xp, \
             tc.tile_pool(name="fvo", bufs=3) as fop, \
             tc.tile_pool(name="tmp", bufs=3) as tmpp, \
             tc.tile_pool(name="mx", bufs=4) as mxp, \
             tc.tile_pool(name="ps", bufs=4, space="PSUM") as psp:
            const_t = constp.tile([128, CW], f32)
            nc.sync.dma_start(const_t[:, :], consts_d[:, :])
            texp_t = const_t[:, 0:JL * L].rearrange("p (a b) -> p a b", b=L)
            lhst_t = const_t[:, JL * L:JL * L + G * 128]
            fvprev = const_t[:, JL * L + G * 128:CW]

            prev_fv = fvprev
            for c in range(NCH):
                xt = xp.tile([128, CH * JL], f32)
                nc.sync.dma_start(xt[:, :], xre[:, c * CH * JL:(c + 1) * CH * JL])
                fvo = fop.tile([128, CH * JL], f32)
                for s in range(CH):
                    rhs = prev_fv
                    ps = psp.tile([128, L], f32)
                    for g in range(G):
                        nc.tensor.matmul(
                            ps[:, g * JL:(g + 1) * JL],
                            const_t[:, JL * L + g * 128:JL * L + (g + 1) * 128],
                            rhs,
                            start=True,
                            stop=True,
                        )
                    # broadcast-read the [128, 64] psum as [128, 16, 64]
                    ps_ap = ps[:, :]
                    ps_bc = bass.AP(
                        tensor=ps_ap.tensor,
                        offset=ps_ap.offset,
                        ap=[ps_ap.ap[0], [0, JL], ps_ap.ap[1]],
                    )
                    tmp = tmpp.tile([128, JL, L], f32)
                    nc.vector.tensor_tensor(tmp[:, :, :], ps_bc, texp_t, op=AluOpType.add)
                    mx = mxp.tile([128, JL], f32)
                    nc.vector.tensor_reduce(
                        mx[:, :], tmp[:, :, :], axis=mybir.AxisListType.X, op=AluOpType.max
                    )
                    nc.vector.tensor_tensor(
                        fvo[:, s * JL:(s + 1) * JL], mx[:, :],
                        xt[:, s * JL:(s + 1) * JL], op=AluOpType.add,
                    )
                    prev_fv = fvo[:, s * JL:(s + 1) * JL]
                nc.sync.dma_start(fvout_d[:, c * CH * JL:(c + 1) * CH * JL], fvo[:, :])
    nc.finalize()
    return nc


def _const_inputs():
    Tr = _CACHE["transition"]
    # texp[g*32+b, jl*64+i] = T[i, g*16+jl]
    texp = np.broadcast_to(
        Tr.T.reshape(G, JL, L).reshape(G, 1, JL * L), (G, BC, JL * L)
    ).reshape(128, JL * L).astype(np.float32).copy()
    # lhst[k, g*128+m] = 1 if k == g*32 + (m % 32)
    lhst = np.zeros((128, G * 128), np.float32)
    for g in range(G):
        for m in range(128):
            lhst[g * BC + (m % BC), g * 128 + m] = 1.0
    init = np.full((L,), NEG, np.float32)
    init[BOS] = 0.0
    fvinit = np.broadcast_to(
        init.reshape(G, 1, JL), (G, BC, JL)
    ).reshape(128, JL).astype(np.float32).copy()
    return np.concatenate([texp, lhst, fvinit], axis=1)


def kernel(X, transition):
    global LAST_EXEC_NS
    from concourse.bass_utils import run_bass_kernel_spmd

    X = np.ascontiguousarray(np.asarray(X, np.float32))
    Tr = np.ascontiguousarray(np.asarray(transition, np.float32))
    _CACHE["transition"] = Tr

    if "nc" not in _CACHE:
        _CACHE["nc"] = _build_bass()
    nc = _CACHE["nc"]

    consts = _const_inputs()
    in_maps = []
    for c in range(NCORES):
        Xc = X[c * BC:(c + 1) * BC]  # [32, 1024, 64]
        xre = (
            Xc.reshape(BC, T, G, JL).transpose(2, 0, 1, 3).reshape(128, T * JL)
        ).astype(np.float32).copy()
        in_maps.append({"xre": xre, "consts": consts})

    res = run_bass_kernel_spmd(nc, in_maps, core_ids=list(range(NCORES)))
    LAST_EXEC_NS = res.exec_time_ns

    # fv_all[b_global, t, j]
    fv = np.empty((B, T, L), np.float32)
    for c in range(NCORES):
        out = np.asarray(res.results[c]["fvout"]).reshape(G, BC, T, JL)
        fv[c * BC:(c + 1) * BC] = out.transpose(1, 2, 0, 3).reshape(BC, T, L)

    # Host backtrace (bit-exact float32, matches jnp.argmax first-index ties)
    term = fv[:, T - 1, :] + Tr[:, EOS][None, :]
    last = term.argmax(axis=1)
    scores = term[np.arange(B), last].astype(np.float32)
    paths = np.empty((B, T), np.int32)
    paths[:, T - 1] = last
    for t in range(T - 1, 0, -1):
        cand = fv[:, t - 1, :] + Tr[:, paths[:, t]].T  # [B, L] float32
        paths[:, t - 1] = cand.argmax(axis=1)
    return scores, paths


# revision 12
# speedup vs baseline: 1616.9192x; 1616.9192x over previous
"""Batched CRF Viterbi decode on 8 Trainium2 NeuronCores.

Strategy (data parallel, 32 sentences/core):
  Device forward pass computes the Viterbi DP table fv[t] for every step
  (the O(B*T*L^2) part) and streams it to HBM.  Per step:
    - 4 tiny PE matmuls all-gather fv across the 4 partition j-groups into
      PSUM (broadcast fv[b, :] to every partition that needs it) .
    - one DVE tensor_tensor add: tmp[(g,b),(jl,i)] = fv[b,i] + T[i, g*16+jl]
    - one DVE segmented reduce_max over i -> mx[(g,b), jl]
    - one tiny DVE add of the emission -> fv_t
  Host does the O(B*T*L) backtrace (argmax recompute along the path),
  bit-exact in float32 so paths match the reference argmax tie-breaks.

Layout per core: partition p = g*32 + b  (g = j-group of 16 labels,
b = sentence), free dim inside a step = (jl, i) = (16, 64).
"""

import sys

import numpy as np

for _p in ("/opt/trn_rl_repo",):
    if _p not in sys.path:
        sys.path.insert(0, _p)

B, T, L = 256, 1024, 64
NCORES = 8
BC = B // NCORES  # 32 sentences per core
G, JL = 4, 16  # 4 j-groups x 16 labels
NEG = -10000.0
BOS, EOS = 62, 63
CH = 64  # steps per DMA chunk
NCH = T // CH

_CACHE = {}
LAST_EXEC_NS = None


def _build_bass():
    import concourse.bass as bass
    import concourse.bacc as bacc
    import concourse.mybir as mybir
    from concourse.mybir import AluOpType
    from concourse.tile import TileContext

    f32 = mybir.dt.float32
    nc = bacc.Bacc(None, target_bir_lowering=False)

    xre = nc.dram_tensor("xre", [128, T * JL], f32, kind="ExternalInput")
    # consts concatenated so ONE DMA (one semaphore) loads them all:
    # [texp | lhst | fvinit] = [1024 | 512 | 16]
    CW = JL * L + G * 128 + JL
    consts_d = nc.dram_tensor("consts", [128, CW], f32, kind="ExternalInput")
    fvout_d = nc.dram_tensor("fvout", [128, T * JL], f32, kind="ExternalOutput")

    with TileContext(nc) as tc:
        with tc.tile_pool(name="const", bufs=1) as constp, \
             tc.tile_pool(name="xin", bufs=3) as xp, \
             tc.tile_pool(name="fvo", bufs=3) as fop, \
             tc.tile_pool(name="tmp", bufs=4) as tmpp, \
             tc.tile_pool(name="mx", bufs=6) as mxp, \
             tc.tile_pool(name="ps", bufs=7, space="PSUM") as psp:
            const_t = constp.tile([128, CW], f32)
            nc.sync.dma_start(const_t[:, :], consts_d[:, :])
            texp_t = const_t[:, 0:JL * L].rearrange("p (a b) -> p a b", b=L)
            lhst_t = const_t[:, JL * L:JL * L + G * 128]
            fvprev = const_t[:, JL * L + G * 128:CW]

            prev_fv = fvprev
            for c in range(NCH):
                xt = xp.tile([128, CH * JL], f32)
                nc.sync.dma_start(xt[:, :], xre[:, c * CH * JL:(c + 1) * CH * JL])
                fvo = fop.tile([128, CH * JL], f32)
                for s in range(CH):
                    rhs = prev_fv
                    ps = psp.tile([128, L], f32)
                    for g in range(G):
                        nc.tensor.matmul(
                            ps[:, g * JL:(g + 1) * JL],
                            const_t[:, JL * L + g * 128:JL * L + (g + 1) * 128],
                            rhs,
                            start=True,
                            stop=True,
                        )
                    # broadcast-read the [128, 64] psum as [128, 16, 64]
                    ps_ap = ps[:, :]
                    ps_bc = bass.AP(
                        tensor=ps_ap.tensor,
                        offset=ps_ap.offset,
                        ap=[ps_ap.ap[0], [0, JL], ps_ap.ap[1]],
                    )
                    tmp = tmpp.tile([128, JL, L], f32)
                    nc.vector.tensor_tensor(tmp[:, :, :], ps_bc, texp_t, op=AluOpType.add)
                    mx = mxp.tile([128, JL], f32)
                    nc.vector.tensor_reduce(
                        mx[:, :], tmp[:, :, :], axis=mybir.AxisListType.X, op=AluOpType.max
                    )
                    nc.vector.tensor_tensor(
                        fvo[:, s * JL:(s + 1) * JL], mx[:, :],
                        xt[:, s * JL:(s + 1) * JL], op=AluOpType.add,
                    )
                    prev_fv = fvo[:, s * JL:(s + 1) * JL]
                nc.sync.dma_start(fvout_d[:, c * CH * JL:(c + 1) * CH * JL], fvo[:, :])
    nc.finalize()
    return nc


def _const_inputs():
    Tr = _CACHE["transition"]
    # texp[g*32+b, jl*64+i] = T[i, g*16+jl]
    texp = np.broadcast_to(
        Tr.T.reshape(G, JL, L).reshape(G, 1, JL * L), (G, BC, JL * L)
    ).reshape(128, JL * L).astype(np.float32).copy()
    # lhst[k, g*128+m] = 1 if k == g*32 + (m % 32)
    lhst = np.zeros((128, G * 128), np.float32)
    for g in range(G):
        for m in range(128):
            lhst[g * BC + (m % BC), g * 128 + m] = 1.0
    init = np.full((L,), NEG, np.float32)
    init[BOS] = 0.0
    fvinit = np.broadcast_to(
        init.reshape(G, 1, JL), (G, BC, JL)
    ).reshape(128, JL).astype(np.float32).copy()
    return np.concatenate([texp, lhst, fvinit], axis=1)


def kernel(X, transition):
    global LAST_EXEC_NS
    from concourse.bass_utils import run_bass_kernel_spmd

    X = np.ascontiguousarray(np.asarray(X, np.float32))
    Tr = np.ascontiguousarray(np.asarray(transition, np.float32))
    _CACHE["transition"] = Tr

    if "nc" not in _CACHE:
        _CACHE["nc"] = _build_bass()
    nc = _CACHE["nc"]

    consts = _const_inputs()
    in_maps = []
    for c in range(NCORES):
        Xc = X[c * BC:(c + 1) * BC]  # [32, 1024, 64]
        xre = (
            Xc.reshape(BC, T, G, JL).transpose(2, 0, 1, 3).reshape(128, T * JL)
        ).astype(np.float32).copy()
        in_maps.append({"xre": xre, "consts": consts})

    import os

    trace = bool(int(os.environ.get("CRF_TRACE", "0")))
    res = run_bass_kernel_spmd(
        nc, in_maps, core_ids=list(range(NCORES)), trace=trace,
        tmpdir=os.environ.get("CRF_TRACE_DIR") or None,
    )
    LAST_EXEC_NS = res.exec_time_ns

    # fv_all[b_global, t, j]
    fv = np.empty((B, T, L), np.float32)
    for c in range(NCORES):
        out = np.asarray(res.results[c]["fvout"]).reshape(G, BC, T, JL)
        fv[c * BC:(c + 1) * BC] = out.transpose(1, 2, 0, 3).reshape(BC, T, L)

    # Host backtrace (bit-exact float32, matches jnp.argmax first-index ties)
    term = fv[:, T - 1, :] + Tr[:, EOS][None, :]
    last = term.argmax(axis=1)
    scores = term[np.arange(B), last].astype(np.float32)
    paths = np.empty((B, T), np.int32)
    paths[:, T - 1] = last
    for t in range(T - 1, 0, -1):
        cand = fv[:, t - 1, :] + Tr[:, paths[:, t]].T  # [B, L] float32
        paths[:, t - 1] = cand.argmax(axis=1)
    return scores, paths
